# revision 6
# baseline (speedup 1.0000x reference)
"""Trainium2 Bass kernel for nn_BiomechanicsLoss_kdtree.

Computes norm(diag(et @ C @ et.T)) / n_valid where et is the strain tensor
built from nearest-inside-neighbor deltas (brute-force KNN over N=12288 pts).

Device strategy (8 NeuronCores, SPMD — same NEFF, different data):
  * Only INSIDE rows matter (valid subsets inside) and only INSIDE points are
    candidates, so the distance problem shrinks from N^2 to M^2 (M ~ N/2).
  * Queries = inside points in compacted order, padded to 128*T*8 slots and
    row-sharded across the 8 cores (QC = 128*T per core).
  * Candidates = the same compacted inside set as a [4, FD] table
    [cx; cy; cz; -|c|^2], padded with -BIG columns; per-core the table is
    np.roll()'d by -core*QC so each query tile's self-match sits on a static
    diagonal -> self-exclusion is one [128,128] "-BIG eye" add, identical on
    every core (no per-core control flow).
  * Per query tile [128 rows]: PE computes scores s = 2*q.w - |c|^2 (argmax s
    == argmin distance) with K=4 float32r matmuls into PSUM; ACT copies PSUM
    into a [128, FD] SBUF row block; DVE applies the diag mask then runs
    max8 + max_index to get the argmax column per row.
  * Host maps rotated local indices back to global ids and runs the O(N)
    strain/quadratic-form tail in float64 (matches fp32 reference to ~1e-7).
"""

import os
import numpy as np

NCORES = 8
BIG = np.float32(1.0e30)

# set by kernel() when trace=True is requested (see test.py)
LAST_EXEC_TIME_NS = None
LAST_PROFILE = None

_PROGRAM_CACHE = {}


def _build_program(QC, T, FD):
    """Build the per-core Bass/Tile program (identical for all cores)."""
    import concourse.bacc as bacc
    import concourse.mybir as mybir
    from concourse import tile

    f32 = mybir.dt.float32
    u32 = mybir.dt.uint32
    f32r = mybir.dt.float32r

    # Bacc (not raw Bass): its compile() pipeline moves/splits semaphore
    # waits to satisfy the TRN2 1-wait-per-instruction constraint.
    nc = bacc.Bacc(trn_type="TRN2", target_bir_lowering=False, debug=False)
    lhsT_d = nc.dram_tensor("lhsT", [4, QC], f32, kind="ExternalInput")
    rhs_d = nc.dram_tensor("rhs", [4, FD], f32, kind="ExternalInput")
    eye_d = nc.dram_tensor("negeye", [128, 128], f32, kind="ExternalInput")
    idx_d = nc.dram_tensor("idx_out", [128, T], u32, kind="ExternalOutput")
    val_d = nc.dram_tensor("val_out", [128, T], f32, kind="ExternalOutput")

    CH = 2048  # PSUM staging chunk (4 banks); FD must be a multiple of 512

    with tile.TileContext(nc) as tc:
        with tc.tile_pool(name="const", bufs=1) as cpool, \
             tc.tile_pool(name="rows", bufs=2) as rpool, \
             tc.tile_pool(name="ps", bufs=2, space="PSUM") as ppool, \
             tc.tile_pool(name="small", bufs=2) as spool:
            lhsT = cpool.tile_from(lhsT_d[:, :])
            rhs = cpool.tile_from(rhs_d[:, :])
            eye = cpool.tile_from(eye_d[:, :])
            idx_sb = cpool.tile([128, T], u32)
            val_sb = cpool.tile([128, T], f32)
            # Round the matmul operands to float32r via DVE copies. This (a)
            # satisfies the BIR verifier (fp32r consumers need a rounding
            # producer, a plain DMA is not one), (b) runs the PE at 1
            # cycle/row instead of fp32's 4, and (c) consolidates the input
            # DMA waits so the self-weight-loading matmuls only carry a
            # single engine-sem wait (HW limits sync waits per instruction).
            lr = cpool.tile([4, QC], f32r)
            rr = cpool.tile([4, FD], f32r)
            nc.vector.tensor_copy(lr[:], lhsT[:])
            nc.vector.tensor_copy(rr[:], rhs[:])
            for t in range(T):
                srow = rpool.tile([128, FD], f32, tag="srow")
                for base in range(0, FD, CH):
                    width = min(CH, FD - base)
                    ps = ppool.tile([128, CH], f32, tag="ps")
                    for k in range(0, width, 512):
                        nc.tensor.matmul(
                            ps[:, k:k + 512],
                            lr[:, t * 128:(t + 1) * 128],
                            rr[:, base + k:base + k + 512],
                            start=True, stop=True,
                        )
                    nc.scalar.copy(srow[:, base:base + width], ps[:, :width])
                # self-exclusion: query slot (t*128+p) lives at rotated
                # candidate column (t*128+p) -> add -BIG on that diagonal
                nc.vector.tensor_tensor(
                    out=srow[:, t * 128:(t + 1) * 128],
                    in0=srow[:, t * 128:(t + 1) * 128],
                    in1=eye[:, :],
                    op=mybir.AluOpType.add,
                )
                v8 = spool.tile([128, 8], f32, tag="v8")
                i8 = spool.tile([128, 8], u32, tag="i8")
                nc.vector.max(v8[:], srow[:])
                nc.vector.max_index(i8[:], v8[:], srow[:])
                nc.vector.tensor_copy(idx_sb[:, t:t + 1], i8[:, 0:1])
                nc.vector.tensor_copy(val_sb[:, t:t + 1], v8[:, 0:1])
            nc.sync.dma_start(idx_d[:, :], idx_sb[:])
            nc.sync.dma_start(val_d[:, :], val_sb[:])
    nc.compile()
    return nc


def _c_matrix():
    VP, EP = 0.4, 0.21
    Ci = np.zeros((6, 6), dtype=np.float64)
    Ci[0, 0] = 1 / EP; Ci[0, 1] = -VP / EP; Ci[0, 2] = -VP / EP
    Ci[1, 0] = -VP / EP; Ci[1, 1] = 1 / EP; Ci[1, 2] = -VP / EP
    Ci[2, 0] = -VP; Ci[2, 1] = -VP; Ci[2, 2] = 1 / EP
    Ci[3, 3] = 2 * (1 + VP) / EP
    Ci[4, 4] = 2 * (1 + VP) / EP
    Ci[5, 5] = 2 * (1 + VP) / EP
    # replicate reference: invert in float64, round to float32, then use
    return np.linalg.inv(Ci).astype(np.float32).astype(np.float64)


def kernel(new_xyz, xyz, gt_sdf, trace=False):
    global LAST_EXEC_TIME_NS, LAST_PROFILE
    from concourse.bass_utils import run_bass_kernel_spmd

    w = np.ascontiguousarray(np.asarray(new_xyz, dtype=np.float32))
    xyz = np.ascontiguousarray(np.asarray(xyz, dtype=np.float32))
    gt_sdf = np.asarray(gt_sdf, dtype=np.float32)
    N = w.shape[0]

    inside = gt_sdf < 1e-8
    ins_idx = np.nonzero(inside)[0]
    M = int(len(ins_idx))
    if M == 0:
        return np.float32(np.nan)

    T = -(-(-(-M // 128)) // NCORES)          # query tiles per core
    QC = T * 128                              # queries per core
    QTOT = QC * NCORES                        # padded total query slots
    FD = 512 * (-(-max(M, QTOT) // 512))      # candidate columns (>= QTOT)

    wi = w[ins_idx]                           # [M, 3] compacted inside pts
    sqc = (wi * wi).sum(1).astype(np.float32)

    cand = np.zeros((4, FD), dtype=np.float32)
    cand[0, :M] = wi[:, 0]
    cand[1, :M] = wi[:, 1]
    cand[2, :M] = wi[:, 2]
    cand[3, :M] = -sqc
    cand[3, M:] = -BIG

    wq = np.zeros((QTOT, 3), dtype=np.float32)
    wq[:M] = wi

    negeye = np.zeros((128, 128), dtype=np.float32)
    np.fill_diagonal(negeye, -BIG)

    key = (QC, T, FD)
    if key not in _PROGRAM_CACHE:
        _PROGRAM_CACHE[key] = _build_program(QC, T, FD)
    nc = _PROGRAM_CACHE[key]

    in_maps = []
    for c in range(NCORES):
        lhsT = np.empty((4, QC), dtype=np.float32)
        sl = slice(c * QC, (c + 1) * QC)
        lhsT[0] = 2.0 * wq[sl, 0]
        lhsT[1] = 2.0 * wq[sl, 1]
        lhsT[2] = 2.0 * wq[sl, 2]
        lhsT[3] = 1.0
        in_maps.append({
            "lhsT": lhsT,
            "rhs": np.ascontiguousarray(np.roll(cand, -c * QC, axis=1)),
            "negeye": negeye,
        })

    res = run_bass_kernel_spmd(nc, in_maps, list(range(NCORES)), trace=trace)
    if trace:
        LAST_EXEC_TIME_NS = res.exec_time_ns
        LAST_PROFILE = res

    # decode: core c, tile t, partition p -> query slot c*QC + t*128 + p
    loc = np.zeros(QTOT, dtype=np.int64)
    for c in range(NCORES):
        o = res.results[c]["idx_out"].astype(np.int64)  # [128, T]
        for t in range(T):
            loc[c * QC + t * 128:c * QC + (t + 1) * 128] = (o[:, t] + c * QC) % FD

    compact = loc[:M]
    if compact.max() >= M:
        bad = np.nonzero(compact >= M)[0]
        raise RuntimeError(f"kernel returned out-of-range NN index for rows {bad[:8]}")

    # host tail in float64 (matches the fp32 reference to ~1e-7)
    qrow_g = ins_idx
    nn_g = ins_idx[compact]
    w64 = w.astype(np.float64)
    motion = (w - xyz).astype(np.float64)
    d2 = ((w64[nn_g] - w64[qrow_g]) ** 2).sum(1)
    nn_d = np.sqrt(d2)
    valid = nn_d > 1e-8
    dm = motion[nn_g] - motion[qrow_g]
    dc = w64[nn_g] - w64[qrow_g] + 1e-8
    dm = np.where(valid[:, None], dm, 0.0)
    dc = np.where(valid[:, None], dc, 1.0)
    du, dv, dwz = dm[:, 0], dm[:, 1], dm[:, 2]
    dx, dy, dz = dc[:, 0], dc[:, 1], dc[:, 2]
    et = np.stack([du / dx, dv / dy, dwz / dz,
                   (du / dy + dv / dx) / 2,
                   (du / dz + dwz / dx) / 2,
                   (dwz / dy + dv / dz) / 2], axis=1)
    C = _c_matrix()
    q = np.einsum('ni,ij,nj->n', et, C, et)
    q = np.where(valid, q, 0.0)
    n_valid = float(valid.sum())
    out = np.linalg.norm(q) / n_valid
    return np.float32(out)


# revision 12
# speedup vs baseline: 1.1235x; 1.1235x over previous
"""Trainium2 Bass kernel for nn_BiomechanicsLoss_kdtree.

Computes norm(diag(et @ C @ et.T)) / n_valid where et is the strain tensor
built from nearest-inside-neighbor deltas (brute-force KNN over N=12288 pts).

Device strategy (8 NeuronCores, SPMD — same NEFF, different data):
  * Only INSIDE rows matter (valid subsets inside) and only INSIDE points are
    candidates, so the distance problem shrinks from N^2 to M^2 (M ~ N/2).
  * Queries = inside points in compacted order, padded to 128*T*8 slots and
    row-sharded across the 8 cores (QC = 128*T per core).
  * Candidates = the same compacted inside set as a [4, FD] table
    [cx; cy; cz; -|c|^2], padded with -BIG columns; per-core the table is
    np.roll()'d by -core*QC so each query tile's self-match sits on a static
    diagonal -> self-exclusion is one [128,128] "-BIG eye" add, identical on
    every core (no per-core control flow).
  * Per query tile [128 rows]: PE computes scores s = 2*q.w - |c|^2 (argmax s
    == argmin distance) with K=4 float32r matmuls into PSUM; ACT copies PSUM
    into a [128, FD] SBUF row block; DVE applies the diag mask then runs
    max8 + max_index to get the argmax column per row.
  * Host maps rotated local indices back to global ids and runs the O(N)
    strain/quadratic-form tail in float64 (matches fp32 reference to ~1e-7).
"""

import os
import numpy as np

NCORES = 8
BIG = np.float32(1.0e30)

# set by kernel() when trace=True is requested (see test.py)
LAST_EXEC_TIME_NS = None
LAST_PROFILE = None

_PROGRAM_CACHE = {}


def _build_program(QC, T, FD):
    """Build the per-core Bass/Tile program (identical for all cores)."""
    import concourse.bacc as bacc
    import concourse.mybir as mybir
    from concourse import tile

    f32 = mybir.dt.float32
    u32 = mybir.dt.uint32
    f32r = mybir.dt.float32r
    bf16 = mybir.dt.bfloat16

    # Bacc (not raw Bass): its compile() pipeline moves/splits semaphore
    # waits to satisfy the TRN2 1-wait-per-instruction constraint.
    nc = bacc.Bacc(trn_type="TRN2", target_bir_lowering=False, debug=False)
    lhsT_d = nc.dram_tensor("lhsT", [4, QC], f32, kind="ExternalInput")
    rhs_d = nc.dram_tensor("rhs", [4, FD], f32, kind="ExternalInput")
    eye_d = nc.dram_tensor("negeye", [128, 128], bf16, kind="ExternalInput")
    nb_d = nc.dram_tensor("nbias", [128, T], f32, kind="ExternalInput")
    idx_d = nc.dram_tensor("idx_out", [128, T], u32, kind="ExternalOutput")
    val_d = nc.dram_tensor("val_out", [128, T], f32, kind="ExternalOutput")

    CH = 2048  # PSUM staging chunk (4 banks); FD must be a multiple of 512

    with tile.TileContext(nc) as tc:
        with tc.tile_pool(name="const", bufs=1) as cpool, \
             tc.tile_pool(name="rows", bufs=2) as rpool, \
             tc.tile_pool(name="ps", bufs=2, space="PSUM") as ppool, \
             tc.tile_pool(name="small", bufs=2) as spool:
            lhsT = cpool.tile_from(lhsT_d[:, :])
            rhs = cpool.tile_from(rhs_d[:, :])
            eye = cpool.tile_from(eye_d[:, :])
            nbias = cpool.tile_from(nb_d[:, :])
            idx_sb = cpool.tile([128, T], u32)
            val_sb = cpool.tile([128, T], f32)
            # Round the matmul operands to float32r via DVE copies. This (a)
            # satisfies the BIR verifier (fp32r consumers need a rounding
            # producer, a plain DMA is not one), (b) runs the PE at 1
            # cycle/row instead of fp32's 4, and (c) consolidates the input
            # DMA waits so the self-weight-loading matmuls only carry a
            # single engine-sem wait (HW limits sync waits per instruction).
            lr = cpool.tile([4, QC], f32r)
            rr = cpool.tile([4, FD], f32r)
            nc.vector.tensor_copy(lr[:], lhsT[:])
            nc.vector.tensor_copy(rr[:], rhs[:])
            H1, H2 = FD // 2, FD // 4
            for t in range(T):
                # ACT stages PSUM into SBUF as bf16 with the per-row bias
                # -|w_q|^2 fused in: centered scores s' = -d2 make bf16
                # rounding a ~0.4%-of-d2 perturbation, which only reshuffles
                # near-tie neighbors (validated: ~1e-7 output error).
                srow = rpool.tile([128, FD], bf16, tag="srow")
                for base in range(0, FD, CH):
                    width = min(CH, FD - base)
                    ps = ppool.tile([128, CH], f32, tag="ps")
                    for k in range(0, width, 512):
                        nc.tensor.matmul(
                            ps[:, k:k + 512],
                            lr[:, t * 128:(t + 1) * 128],
                            rr[:, base + k:base + k + 512],
                            start=True, stop=True,
                        )
                    nc.scalar.activation(
                        srow[:, base:base + width], ps[:, :width],
                        mybir.ActivationFunctionType.Identity,
                        bias=nbias[:, t:t + 1], scale=1.0,
                    )
                # self-exclusion: query slot (t*128+p) lives at rotated
                # candidate column (t*128+p) -> add -BIG on that diagonal
                nc.vector.tensor_tensor(
                    out=srow[:, t * 128:(t + 1) * 128],
                    in0=srow[:, t * 128:(t + 1) * 128],
                    in1=eye[:, :],
                    op=mybir.AluOpType.add,
                )
                # bf16 tensor_tensor runs in the DVE 2x mode, so pre-folding
                # the row halves the value-scan cost; the index scan
                # (max_index) still walks the full row for original
                # positions. max preserves the row max and every folded
                # value exists in srow, so the slot-0 lookup is exact.
                h1 = rpool.tile([128, H1], bf16, tag="h1")
                h2 = rpool.tile([128, H2], bf16, tag="h2")
                nc.vector.tensor_tensor(
                    out=h1[:], in0=srow[:, :H1], in1=srow[:, H1:],
                    op=mybir.AluOpType.max)
                nc.vector.tensor_tensor(
                    out=h2[:], in0=h1[:, :H2], in1=h1[:, H2:],
                    op=mybir.AluOpType.max)
                v8 = spool.tile([128, 8], bf16, tag="v8")
                i8 = spool.tile([128, 8], u32, tag="i8")
                nc.vector.max(v8[:], h2[:])
                nc.vector.max_index(i8[:], v8[:], srow[:])
                nc.gpsimd.tensor_copy(idx_sb[:, t:t + 1], i8[:, 0:1])
                nc.vector.tensor_copy(val_sb[:, t:t + 1], v8[:, 0:1])
            nc.sync.dma_start(idx_d[:, :], idx_sb[:])
            nc.sync.dma_start(val_d[:, :], val_sb[:])
    nc.compile()
    return nc


def _c_matrix():
    VP, EP = 0.4, 0.21
    Ci = np.zeros((6, 6), dtype=np.float64)
    Ci[0, 0] = 1 / EP; Ci[0, 1] = -VP / EP; Ci[0, 2] = -VP / EP
    Ci[1, 0] = -VP / EP; Ci[1, 1] = 1 / EP; Ci[1, 2] = -VP / EP
    Ci[2, 0] = -VP; Ci[2, 1] = -VP; Ci[2, 2] = 1 / EP
    Ci[3, 3] = 2 * (1 + VP) / EP
    Ci[4, 4] = 2 * (1 + VP) / EP
    Ci[5, 5] = 2 * (1 + VP) / EP
    # replicate reference: invert in float64, round to float32, then use
    return np.linalg.inv(Ci).astype(np.float32).astype(np.float64)


def kernel(new_xyz, xyz, gt_sdf, trace=False):
    global LAST_EXEC_TIME_NS, LAST_PROFILE
    from concourse.bass_utils import run_bass_kernel_spmd

    w = np.ascontiguousarray(np.asarray(new_xyz, dtype=np.float32))
    xyz = np.ascontiguousarray(np.asarray(xyz, dtype=np.float32))
    gt_sdf = np.asarray(gt_sdf, dtype=np.float32)
    N = w.shape[0]

    inside = gt_sdf < 1e-8
    ins_idx = np.nonzero(inside)[0]
    M = int(len(ins_idx))
    if M == 0:
        return np.float32(np.nan)

    T = -(-(-(-M // 128)) // NCORES)          # query tiles per core
    QC = T * 128                              # queries per core
    QTOT = QC * NCORES                        # padded total query slots
    FD = 512 * (-(-max(M, QTOT) // 512))      # candidate columns (>= QTOT)

    wi = w[ins_idx]                           # [M, 3] compacted inside pts
    sqc = (wi * wi).sum(1).astype(np.float32)

    cand = np.zeros((4, FD), dtype=np.float32)
    cand[0, :M] = wi[:, 0]
    cand[1, :M] = wi[:, 1]
    cand[2, :M] = wi[:, 2]
    cand[3, :M] = -sqc
    cand[3, M:] = -BIG

    wq = np.zeros((QTOT, 3), dtype=np.float32)
    wq[:M] = wi

    import ml_dtypes
    negeye = np.zeros((128, 128), dtype=ml_dtypes.bfloat16)
    np.fill_diagonal(negeye, ml_dtypes.bfloat16(-BIG))

    sqq = np.zeros(QTOT, dtype=np.float32)
    sqq[:M] = sqc

    key = (QC, T, FD)
    if key not in _PROGRAM_CACHE:
        _PROGRAM_CACHE[key] = _build_program(QC, T, FD)
    nc = _PROGRAM_CACHE[key]

    in_maps = []
    for c in range(NCORES):
        lhsT = np.empty((4, QC), dtype=np.float32)
        sl = slice(c * QC, (c + 1) * QC)
        lhsT[0] = 2.0 * wq[sl, 0]
        lhsT[1] = 2.0 * wq[sl, 1]
        lhsT[2] = 2.0 * wq[sl, 2]
        lhsT[3] = 1.0
        # nbias[p, t] = -|w_q|^2 of query slot c*QC + t*128 + p
        nbias = np.ascontiguousarray(
            -sqq[c * QC:(c + 1) * QC].reshape(T, 128).T)
        in_maps.append({
            "lhsT": lhsT,
            "rhs": np.ascontiguousarray(np.roll(cand, -c * QC, axis=1)),
            "negeye": negeye,
            "nbias": nbias,
        })

    res = run_bass_kernel_spmd(nc, in_maps, list(range(NCORES)), trace=trace)
    if trace:
        LAST_EXEC_TIME_NS = res.exec_time_ns
        LAST_PROFILE = res

    # decode: core c, tile t, partition p -> query slot c*QC + t*128 + p
    loc = np.zeros(QTOT, dtype=np.int64)
    for c in range(NCORES):
        o = res.results[c]["idx_out"].astype(np.int64)  # [128, T]
        for t in range(T):
            loc[c * QC + t * 128:c * QC + (t + 1) * 128] = (o[:, t] + c * QC) % FD

    compact = loc[:M]
    if compact.max() >= M:
        bad = np.nonzero(compact >= M)[0]
        raise RuntimeError(f"kernel returned out-of-range NN index for rows {bad[:8]}")

    # host tail in float64 (matches the fp32 reference to ~1e-7)
    qrow_g = ins_idx
    nn_g = ins_idx[compact]
    w64 = w.astype(np.float64)
    motion = (w - xyz).astype(np.float64)
    d2 = ((w64[nn_g] - w64[qrow_g]) ** 2).sum(1)
    nn_d = np.sqrt(d2)
    valid = nn_d > 1e-8
    dm = motion[nn_g] - motion[qrow_g]
    dc = w64[nn_g] - w64[qrow_g] + 1e-8
    dm = np.where(valid[:, None], dm, 0.0)
    dc = np.where(valid[:, None], dc, 1.0)
    du, dv, dwz = dm[:, 0], dm[:, 1], dm[:, 2]
    dx, dy, dz = dc[:, 0], dc[:, 1], dc[:, 2]
    et = np.stack([du / dx, dv / dy, dwz / dz,
                   (du / dy + dv / dx) / 2,
                   (du / dz + dwz / dx) / 2,
                   (dwz / dy + dv / dz) / 2], axis=1)
    C = _c_matrix()
    q = np.einsum('ni,ij,nj->n', et, C, et)
    q = np.where(valid, q, 0.0)
    n_valid = float(valid.sum())
    out = np.linalg.norm(q) / n_valid
    return np.float32(out)


# revision 16
# speedup vs baseline: 1.1792x; 1.0495x over previous
"""Trainium2 Bass kernel for nn_BiomechanicsLoss_kdtree.

Computes norm(diag(et @ C @ et.T)) / n_valid where et is the strain tensor
built from nearest-inside-neighbor deltas (brute-force KNN over N=12288 pts).

Device strategy (8 NeuronCores, SPMD — same NEFF, different data):
  * Only INSIDE rows matter (valid subsets inside) and only INSIDE points are
    candidates, so the distance problem shrinks from N^2 to M^2 (M ~ N/2).
  * Queries = inside points in compacted order, padded to 128*T*8 slots and
    row-sharded across the 8 cores (QC = 128*T per core).
  * Candidates = the same compacted inside set as a [4, FD] table
    [cx; cy; cz; -|c|^2], padded with -BIG columns; per-core the table is
    np.roll()'d by -core*QC so each query tile's self-match sits on a static
    diagonal -> self-exclusion is one [128,128] "-BIG eye" add, identical on
    every core (no per-core control flow).
  * Per query tile [128 rows]: PE computes scores s = 2*q.w - |c|^2 (argmax s
    == argmin distance) with K=4 float32r matmuls into PSUM; ACT copies PSUM
    into a [128, FD] SBUF row block; DVE applies the diag mask then runs
    max8 + max_index to get the argmax column per row.
  * Host maps rotated local indices back to global ids and runs the O(N)
    strain/quadratic-form tail in float64 (matches fp32 reference to ~1e-7).
"""

import os
import numpy as np

NCORES = 8
BIG = np.float32(1.0e30)

# set by kernel() when trace=True is requested (see test.py)
LAST_EXEC_TIME_NS = None
LAST_PROFILE = None

_PROGRAM_CACHE = {}


def _build_program(QC, T, FD):
    """Build the per-core Bass/Tile program (identical for all cores)."""
    import concourse.bacc as bacc
    import concourse.mybir as mybir
    from concourse import tile

    f32 = mybir.dt.float32
    u32 = mybir.dt.uint32
    f32r = mybir.dt.float32r
    bf16 = mybir.dt.bfloat16

    # Bacc (not raw Bass): its compile() pipeline moves/splits semaphore
    # waits to satisfy the TRN2 1-wait-per-instruction constraint.
    nc = bacc.Bacc(trn_type="TRN2", target_bir_lowering=False, debug=False)
    # declared float32r so a plain DMA satisfies the fp32r-producer check
    # (numpy side stays float32 — same bits, PE rounds on read)
    lhsT_d = nc.dram_tensor("lhsT", [4, QC], f32r, kind="ExternalInput")
    rhs_d = nc.dram_tensor("rhs", [4, FD], f32r, kind="ExternalInput")
    eye_d = nc.dram_tensor("negeye", [128, 128], bf16, kind="ExternalInput")
    nb_d = nc.dram_tensor("nbias", [128, T], f32, kind="ExternalInput")
    idx_d = nc.dram_tensor("idx_out", [128, 8 * T], u32, kind="ExternalOutput")
    val_d = nc.dram_tensor("val_out", [128, 8 * T], bf16, kind="ExternalOutput")

    CH = 2048  # PSUM staging chunk (4 banks); FD must be a multiple of 512

    with tile.TileContext(nc) as tc:
        with tc.tile_pool(name="const", bufs=1) as cpool, \
             tc.tile_pool(name="rows", bufs=2) as rpool, \
             tc.tile_pool(name="ps", bufs=2, space="PSUM") as ppool, \
             tc.tile_pool(name="small", bufs=2) as spool:
            lr = cpool.tile_from(lhsT_d[:, :])
            rr = cpool.tile_from(rhs_d[:, :])
            eye = cpool.tile_from(eye_d[:, :])
            nbias = cpool.tile_from(nb_d[:, :])
            idx_sb = cpool.tile([128, 8 * T], u32)
            val_sb = cpool.tile([128, 8 * T], bf16)
            H1, H2 = FD // 2, FD // 4
            for t in range(T):
                # ACT stages PSUM into SBUF as bf16 with the per-row bias
                # -|w_q|^2 fused in: centered scores s' = -d2 make bf16
                # rounding a ~0.4%-of-d2 perturbation, which only reshuffles
                # near-tie neighbors (validated: ~1e-7 output error).
                srow = rpool.tile([128, FD], bf16, tag="srow")
                for base in range(0, FD, CH):
                    width = min(CH, FD - base)
                    ps = ppool.tile([128, CH], f32, tag="ps")
                    for k in range(0, width, 512):
                        nc.tensor.matmul(
                            ps[:, k:k + 512],
                            lr[:, t * 128:(t + 1) * 128],
                            rr[:, base + k:base + k + 512],
                            start=True, stop=True,
                        )
                    nc.scalar.activation(
                        srow[:, base:base + width], ps[:, :width],
                        mybir.ActivationFunctionType.Identity,
                        bias=nbias[:, t:t + 1], scale=1.0,
                    )
                # self-exclusion: query slot (t*128+p) lives at rotated
                # candidate column (t*128+p) -> add -BIG on that diagonal
                nc.vector.tensor_tensor(
                    out=srow[:, t * 128:(t + 1) * 128],
                    in0=srow[:, t * 128:(t + 1) * 128],
                    in1=eye[:, :],
                    op=mybir.AluOpType.add,
                )
                # bf16 tensor_tensor runs in the DVE 2x mode, so pre-folding
                # the row halves the value-scan cost; the index scan
                # (max_index) still walks the full row for original
                # positions. max preserves the row max and every folded
                # value exists in srow, so the slot-0 lookup is exact.
                h1 = rpool.tile([128, H1], bf16, tag="h1")
                h2 = rpool.tile([128, H2], bf16, tag="h2")
                nc.vector.tensor_tensor(
                    out=h1[:], in0=srow[:, :H1], in1=srow[:, H1:],
                    op=mybir.AluOpType.max)
                nc.vector.tensor_tensor(
                    out=h2[:], in0=h1[:, :H2], in1=h1[:, H2:],
                    op=mybir.AluOpType.max)
                # write top-8 values/indices straight into the output arrays
                v8 = val_sb[:, 8 * t:8 * (t + 1)]
                i8 = idx_sb[:, 8 * t:8 * (t + 1)]
                nc.vector.max(v8, h2[:])
                nc.vector.max_index(i8, v8, srow[:])
            nc.sync.dma_start(idx_d[:, :], idx_sb[:])
            nc.sync.dma_start(val_d[:, :], val_sb[:])
    nc.compile()
    return nc


def _c_matrix():
    VP, EP = 0.4, 0.21
    Ci = np.zeros((6, 6), dtype=np.float64)
    Ci[0, 0] = 1 / EP; Ci[0, 1] = -VP / EP; Ci[0, 2] = -VP / EP
    Ci[1, 0] = -VP / EP; Ci[1, 1] = 1 / EP; Ci[1, 2] = -VP / EP
    Ci[2, 0] = -VP; Ci[2, 1] = -VP; Ci[2, 2] = 1 / EP
    Ci[3, 3] = 2 * (1 + VP) / EP
    Ci[4, 4] = 2 * (1 + VP) / EP
    Ci[5, 5] = 2 * (1 + VP) / EP
    # replicate reference: invert in float64, round to float32, then use
    return np.linalg.inv(Ci).astype(np.float32).astype(np.float64)


def kernel(new_xyz, xyz, gt_sdf, trace=False):
    global LAST_EXEC_TIME_NS, LAST_PROFILE
    from concourse.bass_utils import run_bass_kernel_spmd

    w = np.ascontiguousarray(np.asarray(new_xyz, dtype=np.float32))
    xyz = np.ascontiguousarray(np.asarray(xyz, dtype=np.float32))
    gt_sdf = np.asarray(gt_sdf, dtype=np.float32)
    N = w.shape[0]

    inside = gt_sdf < 1e-8
    ins_idx = np.nonzero(inside)[0]
    M = int(len(ins_idx))
    if M == 0:
        return np.float32(np.nan)

    T = -(-(-(-M // 128)) // NCORES)          # query tiles per core
    QC = T * 128                              # queries per core
    QTOT = QC * NCORES                        # padded total query slots
    FD = 512 * (-(-max(M, QTOT) // 512))      # candidate columns (>= QTOT)

    wi = w[ins_idx]                           # [M, 3] compacted inside pts
    sqc = (wi * wi).sum(1).astype(np.float32)

    cand = np.zeros((4, FD), dtype=np.float32)
    cand[0, :M] = wi[:, 0]
    cand[1, :M] = wi[:, 1]
    cand[2, :M] = wi[:, 2]
    cand[3, :M] = -sqc
    cand[3, M:] = -BIG

    wq = np.zeros((QTOT, 3), dtype=np.float32)
    wq[:M] = wi

    import ml_dtypes
    negeye = np.zeros((128, 128), dtype=ml_dtypes.bfloat16)
    np.fill_diagonal(negeye, ml_dtypes.bfloat16(-BIG))

    sqq = np.zeros(QTOT, dtype=np.float32)
    sqq[:M] = sqc

    key = (QC, T, FD)
    if key not in _PROGRAM_CACHE:
        _PROGRAM_CACHE[key] = _build_program(QC, T, FD)
    nc = _PROGRAM_CACHE[key]

    in_maps = []
    for c in range(NCORES):
        lhsT = np.empty((4, QC), dtype=np.float32)
        sl = slice(c * QC, (c + 1) * QC)
        lhsT[0] = 2.0 * wq[sl, 0]
        lhsT[1] = 2.0 * wq[sl, 1]
        lhsT[2] = 2.0 * wq[sl, 2]
        lhsT[3] = 1.0
        # nbias[p, t] = -|w_q|^2 of query slot c*QC + t*128 + p
        nbias = np.ascontiguousarray(
            -sqq[c * QC:(c + 1) * QC].reshape(T, 128).T)
        in_maps.append({
            "lhsT": lhsT,
            "rhs": np.ascontiguousarray(np.roll(cand, -c * QC, axis=1)),
            "negeye": negeye,
            "nbias": nbias,
        })

    res = run_bass_kernel_spmd(nc, in_maps, list(range(NCORES)), trace=trace)
    if trace:
        LAST_EXEC_TIME_NS = res.exec_time_ns
        LAST_PROFILE = res

    # decode: core c, tile t, partition p -> query slot c*QC + t*128 + p
    loc = np.zeros(QTOT, dtype=np.int64)
    for c in range(NCORES):
        o = res.results[c]["idx_out"].astype(np.int64)  # [128, 8*T], slot 0 of 8
        for t in range(T):
            loc[c * QC + t * 128:c * QC + (t + 1) * 128] = (o[:, 8 * t] + c * QC) % FD

    compact = loc[:M]
    if compact.max() >= M:
        bad = np.nonzero(compact >= M)[0]
        raise RuntimeError(f"kernel returned out-of-range NN index for rows {bad[:8]}")

    # host tail in float64 (matches the fp32 reference to ~1e-7)
    qrow_g = ins_idx
    nn_g = ins_idx[compact]
    w64 = w.astype(np.float64)
    motion = (w - xyz).astype(np.float64)
    d2 = ((w64[nn_g] - w64[qrow_g]) ** 2).sum(1)
    nn_d = np.sqrt(d2)
    valid = nn_d > 1e-8
    dm = motion[nn_g] - motion[qrow_g]
    dc = w64[nn_g] - w64[qrow_g] + 1e-8
    dm = np.where(valid[:, None], dm, 0.0)
    dc = np.where(valid[:, None], dc, 1.0)
    du, dv, dwz = dm[:, 0], dm[:, 1], dm[:, 2]
    dx, dy, dz = dc[:, 0], dc[:, 1], dc[:, 2]
    et = np.stack([du / dx, dv / dy, dwz / dz,
                   (du / dy + dv / dx) / 2,
                   (du / dz + dwz / dx) / 2,
                   (dwz / dy + dv / dz) / 2], axis=1)
    C = _c_matrix()
    q = np.einsum('ni,ij,nj->n', et, C, et)
    q = np.where(valid, q, 0.0)
    n_valid = float(valid.sum())
    out = np.linalg.norm(q) / n_valid
    return np.float32(out)
